# revision 1
# baseline (speedup 1.0000x reference)
"""Causal attention (dense transformer block) on 8 Trainium2 NeuronCores.

Problem: x (4, 256, 64, 64) fp32; 1x1-conv q/kv projections; 8-head causal
attention over S = 64*64 = 4096 flattened pixels (head_dim 32); output
projection.  Full inputs in, full output out.

Sharding: 8 cores = 4 batches x 2 head-groups (4 heads each).  Each core
computes q/k/v projections for its head group, flash-style causal attention
(scores kept transposed: k-positions on partitions, q-positions on free dim,
so softmax denominators come out of the AV matmul via an appended ones
column), and a partial output projection.  Host sums the two head-group
partials per batch and adds the output bias.

All matmuls run as float32r (full PE rate at N>=512, near-fp32 precision).
Softmax skips the max-subtraction pass (scores are O(1) here, exp cannot
overflow) and normalizes after the AV matmul.
"""

import math
from contextlib import ExitStack

import numpy as np

import concourse.bass as bass
import concourse.tile as tile
from concourse import bacc, mybir

N_CORES = 8
N, C, HH, WW = 4, 256, 64, 64
S = HH * WW            # 4096
E = 256                # q/k width
O = 256                # v/out width
H = 8                  # heads
HD = E // H            # 32 head dim
HG = 4                 # heads per core
P = 128                # partitions
QC = 512               # q-chunk (matmul moving free dim)
KT = 128               # k-tile (contraction block for AV)
NQ = S // QC           # 8 q-chunks
NEGM = -1.0e5          # additive mask value (exp(-big) == 0)
ACT_W = 3              # k-tiles exp'd per ScalarE call (3 psum banks)

F32 = mybir.dt.float32
F32R = mybir.dt.float32r
BF16 = mybir.dt.bfloat16

# QK scores in bf16: stationary loads 2 cols/cycle (vs ~0.5 for fp32r), the
# single biggest per-matmul cost in the K=32 QK shape.  Rel-err impact is
# ~2e-3 (scores are O(1) logits; softmax renormalizes).
QK_BF16 = False
QK_DT = BF16 if QK_BF16 else F32R


def build_kernel(reps=1):
    nc = bacc.Bacc("TRN2", target_bir_lowering=False, debug=False,
                   num_devices=N_CORES)

    # Per-core inputs (same shapes on every core, different data).
    xf = nc.dram_tensor("xf", (C, S), F32, kind="ExternalInput").ap()
    wqT = nc.dram_tensor("wqT", (C, P), F32, kind="ExternalInput").ap()
    wkT = nc.dram_tensor("wkT", (C, P), F32, kind="ExternalInput").ap()
    wvT = nc.dram_tensor("wvT", (C, O), F32, kind="ExternalInput").ap()
    wpT = nc.dram_tensor("wpT", (2, P, P), F32, kind="ExternalInput").ap()
    bq = nc.dram_tensor("bq", (P, 1), F32, kind="ExternalInput").ap()
    bk = nc.dram_tensor("bk", (P, 1), F32, kind="ExternalInput").ap()
    bv = nc.dram_tensor("bv", (1, P), F32, kind="ExternalInput").ap()
    masks = nc.dram_tensor("masks", (4, P, QC), F32, kind="ExternalInput").ap()
    out = nc.dram_tensor("out", (O, S), F32, kind="ExternalOutput").ap()

    with tile.TileContext(nc) as tc:
        with ExitStack() as ctx:
            _emit(ctx, tc, nc, xf, wqT, wkT, wvT, wpT, bq, bk, bv, masks, out,
                  reps=reps)

    nc.compile()
    return nc


def _emit(ctx, tc, nc, xf, wqT, wkT, wvT, wpT, bq, bk, bv, masks, out,
          reps=1):
    scale = 1.0 / math.sqrt(HD)
    Exp = mybir.ActivationFunctionType.Exp
    Ident = mybir.ActivationFunctionType.Identity

    consts = ctx.enter_context(tc.tile_pool(name="consts", bufs=1))
    qk_ps = ctx.enter_context(tc.tile_pool(name="qk_ps", bufs=2, space="PSUM"))
    av_ps = ctx.enter_context(tc.tile_pool(name="av_ps", bufs=2, space="PSUM"))
    work = ctx.enter_context(tc.tile_pool(name="work", bufs=6))
    norm = ctx.enter_context(tc.tile_pool(name="norm", bufs=2))
    tmp = ctx.enter_context(tc.tile_pool(name="tmp", bufs=1))

    # ---- load constants / weights -------------------------------------
    # DMA-loaded data cannot feed FP32r matmuls directly; a compute-engine
    # copy with float32r output performs the required rounding.
    wq_ld = tmp.tile([P, 2, P], F32, tag="w")
    nc.sync.dma_start(out=wq_ld, in_=wqT.rearrange("(c p) m -> p c m", p=P))
    wq_sb = consts.tile([P, 2, P], F32R)
    nc.vector.tensor_copy(wq_sb, wq_ld)
    wk_ld = tmp.tile([P, 2, P], F32, tag="w")
    nc.sync.dma_start(out=wk_ld, in_=wkT.rearrange("(c p) m -> p c m", p=P))
    wk_sb = consts.tile([P, 2, P], F32R)
    nc.vector.tensor_copy(wk_sb, wk_ld)
    wv_ld = tmp.tile([P, 2, O], F32, tag="w")
    nc.sync.dma_start(out=wv_ld, in_=wvT.rearrange("(c p) m -> p c m", p=P))
    wv_sb = consts.tile([P, 2, O], F32R)
    nc.vector.tensor_copy(wv_sb, wv_ld)
    wp_ld = tmp.tile([P, 2, P], F32, tag="w")
    nc.sync.dma_start(out=wp_ld, in_=wpT.rearrange("m p n -> p m n"))
    wp_sb = consts.tile([P, 2, P], F32R)
    nc.vector.tensor_copy(wp_sb, wp_ld)
    # sliced load+round so the first projection matmuls start after one
    # 512-col slice instead of the whole 4 MB x transfer (~19us startup)
    x_ld = tmp.tile([P, 2, S], F32, tag="big")  # xf as two 128-row chunks
    x_sb = consts.tile([P, 2, S], F32R)
    xr = xf.rearrange("(c p) s -> p c s", p=P)
    for sl in range(NQ):
        nc.sync.dma_start(out=x_ld[:, :, bass.ts(sl, QC)],
                          in_=xr[:, :, bass.ts(sl, QC)])
        nc.vector.tensor_copy(x_sb[:, :, bass.ts(sl, QC)],
                              x_ld[:, :, bass.ts(sl, QC)])
    bq_sb = consts.tile([P, 1], F32)
    nc.sync.dma_start(out=bq_sb, in_=bq)
    bk_sb = consts.tile([P, 1], F32)
    nc.sync.dma_start(out=bk_sb, in_=bk)
    bv_row = consts.tile([1, P], F32)
    nc.sync.dma_start(out=bv_row, in_=bv)
    mask_sb = consts.tile([P, 4, QC], F32)
    nc.sync.dma_start(out=mask_sb, in_=masks.rearrange("m p q -> p m q"))

    bv_bc = consts.tile([P, P], F32)            # bv broadcast down partitions
    nc.gpsimd.partition_broadcast(bv_bc, bv_row)

    # On-device repeat loop for timing runs (reps>1): the whole compute
    # phase re-executes; consts/DMA loads stay outside.
    if reps > 1:
        loop_cm = tc.For_i(0, reps, 1)
        loop_cm.__enter__()

    # ---- q/k projections: qT/kT = W.T-slice @ xf + bias ----------------
    qT = consts.tile([P, S], QK_DT)              # 4 heads x 32 dims on partitions
    kT = consts.tile([P, S], QK_DT)
    for dst, w_sb, b_sb in ((qT, wq_sb, bq_sb), (kT, wk_sb, bk_sb)):
        for j in range(NQ):
            ps = qk_ps.tile([P, 3 * QC], F32, tag="qk")
            for cc in range(2):
                nc.tensor.matmul(ps[:, 0:QC], w_sb[:, cc, :],
                                 x_sb[:, cc, bass.ts(j, QC)],
                                 start=(cc == 0), stop=(cc == 1))
            nc.scalar.activation(dst[:, bass.ts(j, QC)], ps[:, 0:QC],
                                 Ident, bias=b_sb, scale=1.0)

    # ---- v projection, position-major: v[s, o] for our 4 heads ---------
    # One k-tile of 128 positions per matmul; N=256 (all 8 heads) keeps
    # float32r at full rate; we keep only our head-group's 128 columns.
    # v_st[h]: (128 kpos, 34) per k-tile: cols 0:32 = v, col 32 = 1.0.
    v_st = [consts.tile([P, S // KT, 34], F32R, name=f"v_st{h}")
            for h in range(HG)]
    ones2 = consts.tile([P, 2], F32)
    nc.vector.memset(ones2, 1.0)
    ones_bc = bass.AP(tensor=ones2.tensor, offset=ones2.offset,
                      ap=[ones2.ap[0], [0, S // KT], ones2.ap[1]])
    for h in range(HG):
        nc.vector.tensor_copy(v_st[h][:, :, 32:34], ones_bc)
    # wvT columns are pre-rotated on the host so this core's head-group
    # occupies columns 0:128 of the v projection output.
    for st in range(S // KT):
        ps = qk_ps.tile([P, 3 * QC], F32, tag="qk")
        for cc in range(2):
            nc.tensor.matmul(ps[:, 0:O], x_sb[:, cc, bass.ts(st, KT)],
                             wv_sb[:, cc, :], start=(cc == 0),
                             stop=(cc == 1))
        for h in range(HG):
            nc.vector.tensor_add(v_st[h][:, st, 0:32],
                                 ps[:, h * HD:(h + 1) * HD],
                                 bv_bc[:, h * HD:(h + 1) * HD])

    # Matmul operands cannot start at partition 96 (PE quadrant-3 weight
    # feed is unsupported), so head 3's q/k rows get their own partition-0
    # tiles.
    q3k3 = tmp.tile([HD, 2, S], QK_DT, tag="big")
    nc.vector.tensor_copy(q3k3[:, 0, :], qT[3 * HD:4 * HD, :])
    nc.vector.tensor_copy(q3k3[:, 1, :], kT[3 * HD:4 * HD, :])

    # ---- attention ------------------------------------------------------
    # Emission in window-groups of G: G windows of QK+exp(+mask), then those
    # windows' AV matmuls.  Grouping keeps the PE on long runs of one
    # stationary shape (QK vs AV weight loads serialize when alternating),
    # and the one-group skew lets ScalarE exp run concurrently with both.
    # The softmax denominator row is copied out of PSUM immediately so the
    # accumulator bank frees before the (serial, DVE) normalization chain.
    G = 4
    outn = consts.tile([P, S], F32R)             # normalized out^T, 4h x 32dv
    # Pack k-tiles into psum windows by column width (<= 1536).  Diagonal
    # tiles of chunks j>=1 shrink to their causally-valid columns [c0:512]
    # (c0 capped at 256 for full-rate fp32r), packed densely so the exp
    # window is one fully-written contiguous span.
    # Matmul psum writes cannot cross a 512-col bank: full tiles take one
    # bank; the two shrunk (256-wide) diagonal tiles pair into one bank.
    windows = []                                 # (h, j, nkt, [(kt, c0, off)])
    for h in range(HG):
        for j in range(NQ):
            nkt = 4 * j + 4
            cur, nb, prev_half = [], 0, False
            for kt in range(nkt):
                c0 = 256 if (j >= 1 and kt >= 4 * j + 2) else 0
                if c0 and prev_half:
                    cur.append((kt, c0, cur[-1][2] + 256))
                    prev_half = False
                    continue
                if nb == 3:
                    windows.append((h, j, nkt, cur))
                    cur, nb = [], 0
                cur.append((kt, c0, nb * QC))
                nb += 1
                prev_half = bool(c0)
            windows.append((h, j, nkt, cur))
    groups = [windows[g0:g0 + G] for g0 in range(0, len(windows), G)]

    av_tiles = {}

    def emit_qk_grp(grp):
        ess = []
        for (h, j, nkt, tiles) in grp:
            qh = qT[h * HD:(h + 1) * HD, :] if h < 3 else q3k3[:, 0, :]
            kh = kT[h * HD:(h + 1) * HD, :] if h < 3 else q3k3[:, 1, :]
            ps = qk_ps.tile([P, 3 * QC], F32, tag="qk", name="ps")
            width = 0
            for (kt, c0, off) in tiles:
                width = max(width, off + QC - c0)
                nc.tensor.matmul(ps[:, off:off + QC - c0],
                                 kh[:, bass.ts(kt, KT)],
                                 qh[:, j * QC + c0:(j + 1) * QC],
                                 start=True, stop=True)
            es = work.tile([P, ACT_W * QC], F32R, tag="es", name="es", bufs=5)
            nc.scalar.activation(es[:, 0:width], ps[:, 0:width],
                                 Exp, scale=scale)
            # causal mask post-exp (multiply by 0/1): off the QK->exp path
            for (kt, c0, off) in tiles:
                if kt >= 4 * j:
                    sl = es[:, off:off + QC - c0]
                    nc.vector.tensor_mul(sl, sl,
                                         mask_sb[:, kt - 4 * j, c0:QC])
            ess.append(es)
        return ess

    def emit_av_grp(grp, ess):
        for es, (h, j, nkt, tiles) in zip(ess, grp):
            if (h, j) not in av_tiles:
                av_tiles[(h, j)] = av_ps.tile([33, QC], F32, tag="av",
                                              name="av")
            av = av_tiles[(h, j)]
            for (kt, c0, off) in tiles:
                nc.tensor.matmul(av[:, c0:QC], v_st[h][:, kt, 0:33],
                                 es[:, off:off + QC - c0],
                                 start=(kt == 0), stop=(kt == nkt - 1))
            if tiles[-1][0] == nkt - 1:
                # quick PSUM evacuation, then normalize rows 0:32 by row 32.
                avs = norm.tile([32, QC], F32, tag="avs", name="avs")
                nc.vector.tensor_copy(avs, av[0:32, :])
                l0 = norm.tile([1, QC], F32, tag="l0", name="l0")
                nc.vector.tensor_copy(l0, av[32:33, :])
                recip = norm.tile([1, QC], F32, tag="recip", name="recip")
                rscr = norm.tile([1, QC], F32, tag="rscr", name="rscr", bufs=1)
                nc.vector.reciprocal_approx_accurate(recip, l0, rscr)
                rbc = norm.tile([32, QC], F32, tag="rbc", name="rbc")
                nc.gpsimd.partition_broadcast(rbc, recip)
                nc.vector.tensor_mul(outn[h * HD:(h + 1) * HD, bass.ts(j, QC)],
                                     avs, rbc)
                del av_tiles[(h, j)]

    for grp in groups:
        ess = emit_qk_grp(grp)
        emit_av_grp(grp, ess)

    # ---- output projection: out = Wp[:, our 128 cols] @ outn ----------
    for j in range(NQ):
        for m in range(2):
            ps = qk_ps.tile([P, 3 * QC], F32, tag="qk")
            nc.tensor.matmul(ps[:, 0:QC], wp_sb[:, m, :],
                             outn[:, bass.ts(j, QC)],
                             start=True, stop=True)
            ob = work.tile([P, QC], F32, tag="ob", bufs=4)
            nc.scalar.activation(ob, ps[:, 0:QC], Ident, bias=0.0, scale=1.0)
            nc.sync.dma_start(
                out=out.rearrange("(m p) s -> p m s", p=P)[:, m,
                                                           bass.ts(j, QC)],
                in_=ob)

    if reps > 1:
        loop_cm.__exit__(None, None, None)


_BUILT = {}


def _get_built(reps=1):
    if reps not in _BUILT:
        _BUILT[reps] = build_kernel(reps)
    return _BUILT[reps]


def make_in_maps(x, Wq, bq, Wkv, bkv, Wp, bp):
    x = np.asarray(x, dtype=np.float32)
    Wq = np.asarray(Wq, dtype=np.float32)
    bq = np.asarray(bq, dtype=np.float32)
    Wkv = np.asarray(Wkv, dtype=np.float32)
    bkv = np.asarray(bkv, dtype=np.float32)
    Wp = np.asarray(Wp, dtype=np.float32)

    Wk, Wv = Wkv[:E], Wkv[E:]
    bk_, bv_ = bkv[:E], bkv[E:]

    # causal masks in transposed-score orientation (kpos partition, qpos free)
    kk = np.arange(P)[:, None]
    qq = np.arange(QC)[None, :]
    mask_np = np.stack([
        (qq >= d0 + kk).astype(np.float32)
        for d0 in (0, 128, 256, 384)])

    in_maps = []
    for c in range(N_CORES):
        n, hg = c // 2, c % 2
        rows = slice(hg * P, (hg + 1) * P)
        # rotate wvT columns so this core's 128 head columns sit at 0:128
        wvT_c = np.ascontiguousarray(np.roll(Wv.T, -hg * P, axis=1))
        in_maps.append({
            "xf": np.ascontiguousarray(x[n].reshape(C, S)),
            "wqT": np.ascontiguousarray(Wq[rows].T),
            "wkT": np.ascontiguousarray(Wk[rows].T),
            "wvT": wvT_c,
            "wpT": np.ascontiguousarray(
                Wp[:, rows].reshape(2, P, P).transpose(0, 2, 1)),
            "bq": np.ascontiguousarray(bq[rows, None]),
            "bk": np.ascontiguousarray(bk_[rows, None]),
            "bv": np.ascontiguousarray(bv_[None, rows]),
            "masks": mask_np,
        })
    return in_maps


def kernel(x, Wq, bq, Wkv, bkv, Wp, bp, n_heads):
    assert int(n_heads) == H
    bp = np.asarray(bp, dtype=np.float32)

    from concourse.bass_utils import run_bass_kernel_spmd

    nc = _get_built()
    in_maps = make_in_maps(x, Wq, bq, Wkv, bkv, Wp, bp)

    res = run_bass_kernel_spmd(nc, in_maps, core_ids=list(range(N_CORES)))

    outp = np.zeros((N, O, S), np.float32)
    for c in range(N_CORES):
        outp[c // 2] += res.results[c]["out"]
    outp += bp[None, :, None]
    return outp.reshape(N, O, HH, WW)

